# revision 1
# baseline (speedup 1.0000x reference)
"""Trainium2 Bass kernel for nn_EnhancedStableNCA (v2).

Strategy (per core, data-parallel over batch: 4 images/core on 8 cores):
- x state in SBUF as 8 row-chunk tiles [128, 18, 132] bf16 (16 rows + 2 halo
  rows, 128 cols + halo col at 1/130 + alignment pad at 0/131); partition
  p = 32*b + c (b = image-in-core, c = channel).
- The three 3x3 depthwise convs fold into the layer-1 matmul:
  sum_{dy,dx} A[dy,dx] @ x_shift.  dy via DMA'd K=48 y-stack (2 images per
  stack at partition bases 0/64), dx via free AP column offsets; 3 matmuls
  accumulate in PSUM.  Adjacent images use disjoint PE row groups so their
  L1 matmuls overlap on the array.
- PSUM: ps1 [128,1024]x2 (4 banks) + ps2 [128,1024] (2) + ps3 [128,1024]
  (2) = 8 banks.  Evacs use wide FDs to amortize fixed op cost.
- h1 = relu(ps1+b1) on ACT (bias fused).  h2m = relu(ps2)*mask in ONE DVE
  scalar_tensor_tensor (valid since b2==0 per spec; nonzero-b2 fallback
  build splits relu to ACT + mask-mult to DVE).
- L3 col-packed (4 images at col positions 0/32/64/96 of one PSUM tile);
  evac fuses (ps3 + b3) + x in one DVE scalar_tensor_tensor.
- tanh on ACT; final *alive multiply on GPSIMD (keeps DVE free).
- alive maxpool runs on GPSIMD from a compact [32,...] alpha gather;
  mask/alive reach all partitions via DRAM-bounce broadcast DMA.
"""

import numpy as np

CH = 16
HID = 128
B, H, W = 32, 128, 128
STEPS = 8
NCORES = 8
NB = B // NCORES          # images per core
NCHUNK = 8                # row chunks per image
RCH = H // NCHUNK         # rows per chunk (16)
PW = W + 4                # padded row width (132); interior cols 2..129
PR = RCH + 2              # chunk rows incl halo (18)
CC = RCH * W              # compact chunk pixels (2048)
HC = CC // 2              # half-chunk pixels (1024)

KX = np.array([[-1., 0., 1.], [-2., 0., 2.], [-1., 0., 1.]], np.float32) / 8.0
KY = np.array([[-1., -2., -1.], [0., 0., 0.], [1., 2., 1.]], np.float32) / 8.0
KL = np.array([[0.05, 0.2, 0.05], [0.2, -1.0, 0.2], [0.05, 0.2, 0.05]], np.float32)

_BUILD_CACHE = {}


def _build(steps, fuse_mask=True):
    import os
    import concourse.bacc as bacc
    import concourse.bass as bass
    import concourse.tile as tile
    from concourse import mybir

    _dbg = os.environ.get("NCA_DBG", "")

    F32 = mybir.dt.float32
    BF16 = mybir.dt.bfloat16
    FP8 = mybir.dt.float8e4
    AF = mybir.ActivationFunctionType
    OP = mybir.AluOpType
    loop_n = None
    if isinstance(steps, tuple):          # ("loop", N) timing variant
        loop_n = steps[1]
    msteps = loop_n if loop_n is not None else steps

    nc = bacc.Bacc("TRN2", target_bir_lowering=False, debug=False)

    # ---- DRAM I/O ----
    xin = nc.dram_tensor("xin", [NB, CH, H + 2, PW], BF16, kind="ExternalInput")
    m8c = nc.dram_tensor("m8c", [msteps * NCHUNK, CC], FP8, kind="ExternalInput")
    w1t = nc.dram_tensor("w1t", [128, 3, HID], BF16, kind="ExternalInput")
    w2t = nc.dram_tensor("w2t", [HID, HID], BF16, kind="ExternalInput")
    w3t = nc.dram_tensor("w3t", [HID, CH], BF16, kind="ExternalInput")
    b1d = nc.dram_tensor("b1d", [HID, 1], F32, kind="ExternalInput")
    b2d = nc.dram_tensor("b2d", [HID, 1], F32, kind="ExternalInput")
    b3d = nc.dram_tensor("b3d", [128, 1], F32, kind="ExternalInput")
    seedd = nc.dram_tensor("seedd", [1], BF16, kind="ExternalInput")
    xout = nc.dram_tensor("xout", [NB, CH, H, W], F32, kind="ExternalOutput")

    with tile.TileContext(nc) as tc:
        with tc.tile_pool(name="state", bufs=1) as state, \
             tc.tile_pool(name="stkp", bufs=3) as stkp, \
             tc.tile_pool(name="h1p", bufs=5) as h1p, \
             tc.tile_pool(name="h2p", bufs=3) as h2p, \
             tc.tile_pool(name="xnp", bufs=2) as xnp, \
             tc.tile_pool(name="mtp", bufs=3) as mtp, \
             tc.tile_pool(name="atp", bufs=3) as atp, \
             tc.tile_pool(name="mpp", bufs=1) as mpp, \
             tc.tile_pool(name="avp", bufs=2) as avp, \
             tc.tile_pool(name="dram", bufs=2, space="DRAM") as dramp, \
             tc.tile_pool(name="ps1p", bufs=2, space="PSUM") as ps1p, \
             tc.tile_pool(name="ps2p", bufs=1, space="PSUM") as ps2p, \
             tc.tile_pool(name="ps3p", bufs=1, space="PSUM") as ps3p:

            # ---- persistent state ----
            xck = [state.tile([128, PR, PW], BF16, tag=f"xck{k}", name=f"xck{k}")
                   for k in range(NCHUNK)]
            w1s = state.tile([128, 3, HID], BF16, tag="w1s")
            w2s = state.tile([HID, HID], BF16, tag="w2s")
            w3s = state.tile([HID, CH], BF16, tag="w3s")
            b1s = state.tile([HID, 1], F32, tag="b1s")
            b2s = state.tile([HID, 1], F32, tag="b2s")
            b3s = state.tile([128, 1], F32, tag="b3s")

            nc.sync.dma_start(out=w1s, in_=w1t[:])
            nc.sync.dma_start(out=w2s, in_=w2t[:])
            nc.sync.dma_start(out=w3s, in_=w3t[:])
            nc.sync.dma_start(out=b1s, in_=b1d[:])
            nc.sync.dma_start(out=b2s, in_=b2d[:])
            nc.sync.dma_start(out=b3s, in_=b3d[:])

            for k in range(NCHUNK):
                nc.vector.memset(xck[k], 0.0)
                for b in range(NB):
                    nc.sync.dma_start(
                        out=xck[k][32 * b:32 * b + CH, :, :],
                        in_=xin[b, :, k * RCH:k * RCH + PR, :])

            # persistent alive mask [32, CC]; row 4k+b = (chunk k, image b).
            # Written at end of each step from alpha gathered incrementally
            # after each chunk epilogue -> no step-head global barrier.
            av = state.tile([32, CC], FP8, tag="av")

            def emit_alive(ac):
                # threshold-then-dilate (gpsimd-legal: adds + scalar cmp):
                # maxpool3(a) > t  ==  boxsum3(a > t) > 0.5
                ab = mpp.tile([32, PR, PW], BF16, tag="ab")
                nc.gpsimd.tensor_single_scalar(ab, ac, 0.01, OP.is_gt)
                d1 = mpp.tile([32, PR, W], BF16, tag="d1")
                nc.gpsimd.tensor_tensor(
                    d1, ab[:, :, 1:W + 1], ab[:, :, 3:W + 3], OP.add)
                nc.gpsimd.tensor_tensor(
                    d1, d1, ab[:, :, 2:W + 2], OP.add)
                d2 = mpp.tile([32, RCH, W], BF16, tag="d2")
                nc.gpsimd.tensor_tensor(
                    d2, d1[:, 0:RCH], d1[:, 2:RCH + 2], OP.add)
                nc.gpsimd.tensor_tensor(
                    d2, d2, d1[:, 1:RCH + 1], OP.add)
                nc.gpsimd.tensor_single_scalar(
                    av, d2.rearrange("p r w -> p (r w)"), 0.5, OP.is_gt)

            # prologue: alive of the initial state (halos valid from load)
            avd = dramp.tile([32, CC], FP8, tag="avd", name="avd")
            ac0 = mpp.tile([32, PR, PW], BF16, tag="ac")
            for k in range(NCHUNK):
                for b in range(NB):
                    nc.sync.dma_start(
                        out=ac0[4 * k + b:4 * k + b + 1, :, :],
                        in_=xck[k][32 * b + 3:32 * b + 4, :, :])
            emit_alive(ac0)
            nc.sync.dma_start(out=avd, in_=av)

            def step_body(si):
                ac = mpp.tile([32, PR, PW], BF16, tag="ac")

                # ---- chunks ----
                for k in range(NCHUNK):
                    mt = mtp.tile([128, CC], FP8, tag="mt")
                    at = atp.tile([128, CC], FP8, tag="at")
                    if "nobc" in _dbg:
                        nc.vector.memset(mt, 1.0)
                        nc.vector.memset(at, 1.0)
                    else:
                        if isinstance(si, int):
                            msrc = m8c[si * NCHUNK + k:si * NCHUNK + k + 1, :]
                        else:
                            msrc = m8c[bass.ds(si * NCHUNK + k, 1), :]
                        nc.scalar.dma_start(out=mt,
                                            in_=msrc.to_broadcast([128, CC]))
                        for b in range(NB):
                            nc.scalar.dma_start(
                                out=at[32 * b:32 * b + CH, :],
                                in_=avd[4 * k + b:4 * k + b + 1, :]
                                    .to_broadcast([CH, CC]))

                    # ---- y-stacks (two image-pairs) ----
                    stks = []
                    for j in range(2):
                        stk = stkp.tile([128, RCH, PW], BF16, tag="stk")
                        for i in range(2):
                            bb = 32 * (2 * j + i)
                            for d in range(3):
                                nc.sync.dma_start(
                                    out=stk[64 * i + 16 * d:
                                            64 * i + 16 * d + CH, :, :],
                                    in_=xck[k][bb:bb + CH, d:d + RCH, :])
                        stks.append(stk)

                    h1t = []      # per-image h1 [128, CC] bf16
                    xn = xnp.tile([128, CC], F32, tag="xn")
                    xn3 = xn.rearrange("p (r w) -> p r w", r=RCH)

                    def l1_half(i, h):
                        """L1 matmuls + relu evac for image i, col half h."""
                        stk = stks[i // 2]
                        hb = 64 * (i % 2)
                        ps1 = ps1p.tile([128, HC], F32, tag="ps1")
                        for n2 in range(2):
                            r0 = 8 * h + 4 * n2
                            for t in range(3):
                                nc.tensor.matmul(
                                    ps1[:, 512 * n2:512 * n2 + 512],
                                    w1s[hb:hb + 48, t, :],
                                    stk[hb:hb + 48, r0:r0 + 4, t + 1:t + 1 + W],
                                    start=(t == 0), stop=(t == 2))
                        nc.scalar.activation(
                            h1t[i][:, HC * h:HC * h + HC], ps1, AF.Relu,
                            bias=b1s)

                    def l23(i, h, ps3):
                        """L2 + h2-evac(+mask) + L3 for image i, half h."""
                        ps2 = ps2p.tile([128, HC], F32, tag="ps2")
                        for m in range(2):
                            nc.tensor.matmul(
                                ps2[:, 512 * m:512 * m + 512], w2s,
                                h1t[i][:, HC * h + 512 * m:HC * h + 512 * m + 512],
                                start=True, stop=True)
                        h2m = h2p.tile([128, HC], BF16, tag="h2m")
                        if fuse_mask:
                            nc.vector.scalar_tensor_tensor(
                                h2m, ps2, 0.0, mt[:, HC * h:HC * h + HC],
                                op0=OP.max, op1=OP.mult)
                        else:
                            nc.scalar.activation(h2m, ps2, AF.Relu, bias=b2s)
                            nc.vector.tensor_tensor(
                                h2m, h2m, mt[:, HC * h:HC * h + HC], OP.mult)
                        cp = 32 * i
                        for m in range(2):
                            nc.tensor.matmul(
                                ps3[cp:cp + CH, 512 * m:512 * m + 512], w3s,
                                h2m[:, 512 * m:512 * m + 512],
                                start=True, stop=True, tile_position=(0, cp))

                    def xn_evac(h, ps3):
                        """xn[:, half h] = (ps3 + b3) + x  (fused stt)."""
                        xi_h = xck[k][0:112, 1 + 8 * h:1 + 8 * h + 8, 2:2 + W]
                        nc.vector.scalar_tensor_tensor(
                            xn3[0:112, 8 * h:8 * h + 8, :],
                            ps3[0:112, :].rearrange("p (r w) -> p r w", r=8),
                            b3s[0:112, :], xi_h, op0=OP.add, op1=OP.add)

                    for i in range(NB):
                        h1t.append(h1p.tile([128, CC], BF16, tag="h1",
                                            name=f"h1_{i}"))

                    # phase A0: L1 halves h=0 (PE row-pair overlap via i alt)
                    for i in range(NB):
                        l1_half(i, 0)
                    # phase B0 interleaved with A1: L2/L3 h=0 + L1 h=1
                    ps3a = ps3p.tile([128, HC], F32, tag="ps3")
                    for i in range(NB):
                        l23(i, 0, ps3a)
                        l1_half(i, 1)
                    xn_evac(0, ps3a)
                    ps3b = ps3p.tile([128, HC], F32, tag="ps3")
                    for i in range(NB):
                        l23(i, 1, ps3b)
                    xn_evac(1, ps3b)

                    # tanh (ACT, in place) then *alive on GPSIMD -> state
                    if "noepi" in _dbg:
                        continue
                    nc.scalar.activation(xn, xn, AF.Tanh)
                    xi = xck[k][:, 1:RCH + 1, 2:2 + W]
                    at3 = at.rearrange("p (r w) -> p r w", r=RCH)
                    if "nogps" in _dbg:
                        nc.vector.tensor_tensor(xi, xn3, at3, OP.mult)
                    else:
                        nc.gpsimd.tensor_tensor(xi, xn3, at3, OP.mult)
                    if k == 2:      # seed pixel: image row 32 = local row 1
                        for b in range(NB):
                            nc.sync.dma_start(
                                out=xck[2][32 * b + 3:32 * b + CH, 1, 34:35],
                                in_=seedd[:].to_broadcast([CH - 3, 1]))

                    # incremental alpha gather for next step's alive mask
                    for b in range(NB):
                        p = 32 * b + 3
                        nc.sync.dma_start(
                            out=ac[4 * k + b:4 * k + b + 1, 1:RCH + 1, :],
                            in_=xck[k][p:p + 1, 1:RCH + 1, :])
                        if k > 0:
                            # my top halo = prev chunk's last interior row
                            nc.sync.dma_start(
                                out=ac[4 * k + b:4 * k + b + 1, 0:1, :],
                                in_=xck[k - 1][p:p + 1, RCH:RCH + 1, :])
                            # prev chunk's bottom halo = my first interior row
                            nc.sync.dma_start(
                                out=ac[4 * (k - 1) + b:4 * (k - 1) + b + 1,
                                       PR - 1:PR, :],
                                in_=xck[k][p:p + 1, 1:2, :])
                        else:
                            nc.sync.dma_start(
                                out=ac[b:b + 1, 0:1, :],
                                in_=xck[0][p:p + 1, 0:1, :])
                        if k == NCHUNK - 1:
                            nc.sync.dma_start(
                                out=ac[4 * k + b:4 * k + b + 1, PR - 1:PR, :],
                                in_=xck[k][p:p + 1, PR - 1:PR, :])

                emit_alive(ac)
                nc.sync.dma_start(out=avd, in_=av)

                # ---- step end: refresh inter-chunk halo rows ----
                for k in range(NCHUNK - 1):
                    nc.sync.dma_start(out=xck[k + 1][:, 0, :],
                                      in_=xck[k][:, RCH, :])
                    nc.sync.dma_start(out=xck[k][:, RCH + 1, :],
                                      in_=xck[k + 1][:, 1, :])

            if loop_n is not None:
                with tc.For_i(0, loop_n, 1) as si:
                    step_body(si)
            else:
                for si in range(steps):
                    step_body(si)

            # ---- write out (strip padding; cast bf16->f32 via SWDGE) ----
            for k in range(NCHUNK):
                for b in range(NB):
                    nc.gpsimd.dma_start(
                        out=xout[b, :, k * RCH:(k + 1) * RCH, :],
                        in_=xck[k][32 * b:32 * b + CH, 1:RCH + 1, 2:2 + W])

    nc.compile()
    return nc


def _prep_weights(w1, b1, w2, b2, w3, b3):
    import ml_dtypes
    bf = ml_dtypes.bfloat16
    w1 = np.asarray(w1, np.float32)
    W1x, W1gx, W1gy, W1lap = (w1[:, 0:16], w1[:, 16:32],
                              w1[:, 32:48], w1[:, 48:64])
    # A[dy, dx, m, c] = tap weight matrices
    A = (KX[:, :, None, None] * W1gx[None, None] +
         KY[:, :, None, None] * W1gy[None, None] +
         KL[:, :, None, None] * W1lap[None, None])
    A = A.astype(np.float32).copy()
    A[1, 1] += W1x
    # w1t[dy*16+c, t, m] = A[dy, t, m, c]
    blk = A.transpose(0, 3, 1, 2).reshape(48, 3, HID)
    w1t = np.zeros((128, 3, HID), np.float32)
    w1t[0:48] = blk
    w1t[64:112] = blk
    w1ct = np.zeros((128, 3, HID), np.float32)
    w2t = np.ascontiguousarray(np.asarray(w2, np.float32).T)
    w3t = np.ascontiguousarray((0.05 * np.asarray(w3, np.float32)).T)
    b1v = np.asarray(b1, np.float32).reshape(HID, 1)
    b2v = np.asarray(b2, np.float32).reshape(HID, 1)
    b3v = np.zeros((128, 1), np.float32)
    for b in range(NB):
        b3v[32 * b:32 * b + CH, 0] = 0.05 * np.asarray(b3, np.float32)
    return (w1t.astype(bf), w1ct.astype(bf), w2t.astype(bf), w3t.astype(bf),
            b1v, b2v, b3v)


LAST_EXEC_NS = None
LAST_PROFILE_JSON = None
LAST_TRACE_PATH = None


def kernel(x, rand, w1, b1, w2, b2, w3, b3, steps=STEPS):
    import ml_dtypes
    from concourse import mybir
    from concourse.bass_utils import run_bass_kernel_spmd

    import os
    fuse_mask = bool(np.all(np.asarray(b2) == 0.0))
    key = (steps, fuse_mask)
    if key not in _BUILD_CACHE:
        _BUILD_CACHE[key] = _build(steps, fuse_mask=fuse_mask)
    nc = _BUILD_CACHE[key]

    f8 = mybir.dt.np(mybir.dt.float8e4)
    x = np.asarray(x, np.float32)
    rand = np.asarray(rand, np.float32)
    w1t, w1ct, w2t, w3t, b1v, b2v, b3v = _prep_weights(w1, b1, w2, b2, w3, b3)
    m8c = (rand < 0.5).astype(np.float32).reshape(STEPS * NCHUNK, CC)
    if isinstance(steps, tuple):
        reps = -(-steps[1] * NCHUNK // m8c.shape[0])
        m8c = np.tile(m8c, (reps, 1))[:steps[1] * NCHUNK]
    m8c = m8c.astype(f8)

    in_maps = []
    for c in range(NCORES):
        shard = x[c * NB:(c + 1) * NB]                 # [4, 16, H, W]
        xp = np.zeros((NB, CH, H + 2, PW), np.float32)
        xp[:, :, 1:H + 1, 2:2 + W] = shard
        xp = xp.astype(ml_dtypes.bfloat16)
        in_maps.append({
            "xin": xp, "m8c": m8c, "w1t": w1t, "w2t": w2t,
            "w3t": w3t, "b1d": b1v, "b2d": b2v, "b3d": b3v,
            "seedd": np.array([np.tanh(1.0)], ml_dtypes.bfloat16),
        })

    run_kwargs = {}
    if os.environ.get("NCA_TRACE"):
        run_kwargs["trace"] = True
        td = os.environ.get("NCA_TRACE_DIR")
        if td:
            os.makedirs(td, exist_ok=True)
            run_kwargs["tmpdir"] = td
    res = run_bass_kernel_spmd(nc, in_maps, core_ids=list(range(NCORES)),
                               **run_kwargs)
    if os.environ.get("NCA_TRACE"):
        global LAST_EXEC_NS, LAST_PROFILE_JSON, LAST_TRACE_PATH
        LAST_EXEC_NS = res.exec_time_ns
        LAST_PROFILE_JSON = res.profile_json
        if res.instructions_and_trace is not None:
            LAST_TRACE_PATH = res.instructions_and_trace[1]
    out = np.concatenate([r["xout"] for r in res.results], axis=0)
    return out.astype(np.float32)



# revision 2
# speedup vs baseline: 1.9872x; 1.9872x over previous
"""Trainium2 Bass kernel for nn_EnhancedStableNCA (v3).

Strategy (per core, data-parallel over batch: 4 images/core on 8 cores):
- Ping-pong full-state tiles XA/XB [128, 130, 132] bf16 (partition 32b+c,
  tile row = image row + 1, tile col = image col + 2; zero borders).  Each
  step reads X and writes Y: no halo refresh, no intra-step RAW hazards.
- The three 3x3 depthwise convs fold into the layer-1 matmul: per chunk of
  16 image rows a y-stack tile [128,16,132] holds 3 dy-shifted copies of
  2 images (partition 64i+16d+c, one 3-dim DMA per dy); 3 dx-tap matmuls
  (K=48, M=1024) accumulate in PSUM.
- M=1024 matmuls throughout (each writes a [128,1024] f32 PSUM = 2 banks).
- h1 = relu(ps1+b1) on ACT.  h2m = relu(ps2)*mask via one DVE stt (b2==0).
- L3 col-packed (4 images at PE col positions 32i); xn = (ps3+b3)+x via
  DVE stt; xnm = xn*alive on GPSIMD; Y = tanh(xnm) on ACT (valid since
  alive is 0/1: alive*tanh(z) == tanh(alive*z)).
- alive: one whole-step alpha gather DMA (ACT queue, right after the last
  tanh of the previous step) into ac [32=8b+k, 18, 132]; threshold+boxsum
  on DVE -> av [32, 2048]; one broadcast DMA -> at [128, 8, 2048] fp8.
  rand mask mt is one broadcast DMA per step, prefetched a chunk early.
- Epilogue of chunk g runs software-pipelined inside chunk g+1 so ACT's
  relu stream never stalls behind tanh.
"""

import numpy as np

CH = 16
HID = 128
B, H, W = 32, 128, 128
STEPS = 8
NCORES = 8
NB = B // NCORES          # images per core
NCHUNK = 8                # row chunks per image
RCH = H // NCHUNK         # rows per chunk (16)
PW = W + 4                # padded row width (132); interior cols 2..129
PH = H + 2                # padded rows (130); interior rows 1..128
CC = RCH * W              # chunk pixels per image (2048)
HC = CC // 2              # half-chunk pixels (1024)

KX = np.array([[-1., 0., 1.], [-2., 0., 2.], [-1., 0., 1.]], np.float32) / 8.0
KY = np.array([[-1., -2., -1.], [0., 0., 0.], [1., 2., 1.]], np.float32) / 8.0
KL = np.array([[0.05, 0.2, 0.05], [0.2, -1.0, 0.2], [0.05, 0.2, 0.05]], np.float32)

_BUILD_CACHE = {}


def _build(steps, fuse_mask=True):
    import os
    import concourse.bacc as bacc
    import concourse.bass as bass
    import concourse.tile as tile
    from concourse import mybir

    _dbg = os.environ.get("NCA_DBG", "")

    F32 = mybir.dt.float32
    BF16 = mybir.dt.bfloat16
    FP8 = mybir.dt.float8e4
    AF = mybir.ActivationFunctionType
    OP = mybir.AluOpType
    AP = bass.AP
    loop_n = None
    if isinstance(steps, tuple):          # ("loop", N) timing variant
        loop_n = steps[1]
        assert loop_n % 2 == 0, "loop variant needs even step count"
    msteps = loop_n if loop_n is not None else steps

    nc = bacc.Bacc("TRN2", target_bir_lowering=False, debug=False)

    # ---- DRAM I/O ----
    xin = nc.dram_tensor("xin", [NB, CH, PH, PW], BF16, kind="ExternalInput")
    m8c = nc.dram_tensor("m8c", [msteps * NCHUNK, CC], FP8, kind="ExternalInput")
    w1t = nc.dram_tensor("w1t", [128, 3, HID], BF16, kind="ExternalInput")
    w2t = nc.dram_tensor("w2t", [HID, HID], BF16, kind="ExternalInput")
    w3t = nc.dram_tensor("w3t", [HID, 2 * CH], BF16, kind="ExternalInput")
    b1d = nc.dram_tensor("b1d", [HID, 1], F32, kind="ExternalInput")
    b2d = nc.dram_tensor("b2d", [HID, 1], F32, kind="ExternalInput")
    b3d = nc.dram_tensor("b3d", [128, 1], F32, kind="ExternalInput")
    seedd = nc.dram_tensor("seedd", [1], BF16, kind="ExternalInput")
    xout = nc.dram_tensor("xout", [NB, CH, H, W], F32, kind="ExternalOutput")

    XEL = PH * PW            # state tile free elems per partition (17160)
    ACEL = (RCH + 2) * PW    # ac tile free elems (2376)
    SEEDV = float(np.tanh(1.0))

    with tile.TileContext(nc) as tc:
        with tc.tile_pool(name="state", bufs=1) as state, \
             tc.tile_pool(name="stkp", bufs=4) as stkp, \
             tc.tile_pool(name="h1p", bufs=10) as h1p, \
             tc.tile_pool(name="h2p", bufs=3) as h2p, \
             tc.tile_pool(name="xnp", bufs=2) as xnp, \
             tc.tile_pool(name="xmp", bufs=2) as xmp, \
             tc.tile_pool(name="mtp", bufs=2) as mtp, \
             tc.tile_pool(name="atp", bufs=1) as atp, \
             tc.tile_pool(name="acp", bufs=1) as acp, \
             tc.tile_pool(name="avp", bufs=2) as avp, \
             tc.tile_pool(name="mpp", bufs=1) as mpp, \
             tc.tile_pool(name="dram", bufs=2, space="DRAM") as dramp, \
             tc.tile_pool(name="ps1p", bufs=2, space="PSUM") as ps1p, \
             tc.tile_pool(name="ps2p", bufs=1, space="PSUM") as ps2p, \
             tc.tile_pool(name="ps3p", bufs=1, space="PSUM") as ps3p:

            # ---- persistent state ----
            XA = state.tile([128, PH, PW], BF16, tag="XA", name="XA")
            XB = state.tile([128, PH, PW], BF16, tag="XB", name="XB")
            w1s = state.tile([128, 3, HID], BF16, tag="w1s")
            w2s = state.tile([HID, HID], BF16, tag="w2s")
            w3s = state.tile([HID, 2 * CH], BF16, tag="w3s")
            b1s = state.tile([HID, 1], F32, tag="b1s")
            b2s = state.tile([HID, 1], F32, tag="b2s")
            b3s = state.tile([128, 1], F32, tag="b3s")

            nc.sync.dma_start(out=w1s, in_=w1t[:])
            nc.sync.dma_start(out=w2s, in_=w2t[:])
            nc.sync.dma_start(out=w3s, in_=w3t[:])
            nc.sync.dma_start(out=b1s, in_=b1d[:])
            nc.sync.dma_start(out=b2s, in_=b2d[:])
            nc.sync.dma_start(out=b3s, in_=b3d[:])

            nc.vector.memset(XA, 0.0)
            nc.vector.memset(XB, 0.0)
            for b in range(NB):
                nc.sync.dma_start(out=XA[32 * b:32 * b + CH, :, :],
                                  in_=xin[b, :, :, :])

            def mt_bcast(si, si_pair=None):
                """rand mask -> mt [128, 8, 2048] fp8, 1 DMA (ACT queue)."""
                mt = mtp.tile([128, NCHUNK, CC], FP8, tag="mt")
                if "nobc" in _dbg:
                    nc.vector.memset(mt, 1.0)
                    return mt
                if si_pair is None:
                    msrc = m8c[si * NCHUNK:(si + 1) * NCHUNK, :]
                else:
                    li, j = si_pair
                    msrc = m8c[bass.ds(NCHUNK * li + NCHUNK * j,
                                       NCHUNK), :]
                nc.scalar.dma_start(
                    out=mt,
                    in_=msrc.unsqueeze(0).to_broadcast([128, NCHUNK, CC]))
                return mt

            def alive_chain(X):
                """gather alpha from X (ACT queue DMA), threshold+boxsum on
                DVE, broadcast to at [128, 8, 2048] (DVE queue DMA)."""
                at = atp.tile([128, NCHUNK, CC], FP8, tag="at")
                if "nobc" in _dbg:
                    nc.vector.memset(at, 1.0)
                    return at
                ac = acp.tile([32, RCH + 2, PW], BF16, tag="ac")
                for b in range(NB):
                    nc.scalar.dma_start(
                        out=ac[NCHUNK * b:NCHUNK * b + NCHUNK, :, :],
                        in_=AP(X[:].tensor, (32 * b + 3) * XEL,
                               [[XEL, 1], [RCH * PW, NCHUNK], [1, ACEL]]))
                ab = mpp.tile([32, RCH + 2, PW], BF16, tag="ab")
                nc.vector.tensor_single_scalar(ab, ac, 0.01, OP.is_gt)
                d1 = mpp.tile([32, RCH + 2, W], BF16, tag="d1")
                nc.vector.tensor_tensor(
                    d1, ab[:, :, 1:W + 1], ab[:, :, 3:W + 3], OP.add)
                nc.vector.tensor_tensor(d1, d1, ab[:, :, 2:W + 2], OP.add)
                d2 = mpp.tile([32, RCH, W], BF16, tag="d2")
                nc.vector.tensor_tensor(
                    d2, d1[:, 0:RCH], d1[:, 2:RCH + 2], OP.add)
                nc.vector.tensor_tensor(d2, d2, d1[:, 1:RCH + 1], OP.add)
                av = avp.tile([32, CC], FP8, tag="av")
                nc.vector.tensor_single_scalar(
                    av, d2.rearrange("p r w -> p (r w)"), 0.5, OP.is_gt)
                # broadcast av -> at via DRAM bounce (partition replication
                # needs a linear source): at[32b+c, k, pix] = av[8b+k, pix]
                avd = dramp.tile([32, CC], FP8, tag="avd", name="avd")
                nc.gpsimd.dma_start(out=avd, in_=av)
                for b in range(NB):
                    nc.gpsimd.dma_start(
                        out=at[32 * b:32 * b + 32, :, :],
                        in_=avd[NCHUNK * b:NCHUNK * b + NCHUNK, :]
                            .rearrange("k c -> (k c)").unsqueeze(0)
                            .to_broadcast([32, NCHUNK * CC])
                            .rearrange("c (k p) -> c k p", k=NCHUNK))
                return at

            def epilogue(ep):
                """Pipelined chunk epilogue: xnm = xn*alive (GPSIMD), then
                Y = tanh(xnm) (ACT) + seed DMAs for chunk 2; on the final
                step also stream this chunk's rows to DRAM."""
                Y, k, xn, at, final = ep
                xnm = xmp.tile([128, CC], BF16, tag="xnm")
                at3 = at[:, k, :].rearrange("p (r w) -> p r w", r=RCH)
                xn3 = xn.rearrange("p (r w) -> p r w", r=RCH)
                if "nogps" in _dbg:
                    nc.vector.tensor_tensor(xnm[0:112], xn3[0:112].rearrange(
                        "p r w -> p (r w)"), at[0:112, k, :], OP.mult)
                else:
                    nc.gpsimd.tensor_tensor(
                        xnm.rearrange("p (r w) -> p r w", r=RCH)[0:112],
                        xn3[0:112], at3[0:112], OP.mult)
                yi = Y[0:112, 16 * k + 1:16 * k + 1 + RCH, 2:2 + W]
                nc.scalar.activation(
                    yi, xnm.rearrange("p (r w) -> p r w", r=RCH)[0:112],
                    AF.Tanh)
                if k == 2:      # seed: image (32,32) = tile (33,34)
                    for b in range(NB):
                        nc.scalar.dma_start(
                            out=Y[32 * b + 3:32 * b + CH, 33:34, 34:35],
                            in_=seedd[:].to_broadcast([CH - 3, 1]))
                if final:
                    for b in range(NB):
                        nc.gpsimd.dma_start(
                            out=xout[b, :, 16 * k:16 * k + RCH, :],
                            in_=Y[32 * b:32 * b + CH,
                                  16 * k + 1:16 * k + 1 + RCH, 2:2 + W])

            # ---------------- software-pipelined emission ----------------
            # Iteration k emits: B(k) [L2/h2ev/L3 h=0 + trio h=1],
            # xn_evac(0,k), stk-prefetch(k+1), epilogue(k-1), C(k)+A(k+1)
            # merged per image [L2(i,1); trio(i,0,k+1); h2ev; L3(i,1)],
            # xn_evac(1,k).  A(k+1) trios cover h2ev latency so PE never
            # stalls between L2 and L3; the PE stream also crosses step
            # boundaries seamlessly (A of chunk 0 next step is emitted in
            # iteration 7 of this step).
            pend = []     # pending epilogue args
            mt_next = []  # prefetched mask tile
            PWS = PW + 2               # padded stk row (no AP merges)
            SPIT = RCH * PWS           # stk partition pitch

            def build_stks(k, X):
                stks = []
                for j in range(2):
                    stk = stkp.tile([128, RCH, PWS], BF16, tag="stk")
                    for i in range(2):
                        for d in range(3):
                            nc.sync.dma_start(
                                out=AP(stk[:].tensor,
                                       (64 * i + 16 * d) * SPIT,
                                       [[SPIT, CH], [1, RCH * PW]]),
                                in_=AP(X[:].tensor,
                                       (64 * j + 32 * i) * XEL
                                       + (16 * k + d) * PW,
                                       [[XEL, CH], [1, RCH * PW]]))
                    stks.append(stk)
                return stks

            def trio(i, h, stks):
                """L1 for image i, half h: 6 matmuls + relu evac.
                Returns the [128, 1024] bf16 h1-half tile."""
                stkf = stks[i // 2].rearrange("p r w -> p (r w)")
                stk3 = AP(stkf.tensor, 0,
                          [[RCH * PWS, 128], [PW, RCH], [1, PW]])
                hb = 64 * (i % 2)
                ps1 = ps1p.tile([128, HC], F32, tag="ps1")
                for n2 in range(2):
                    r0 = 8 * h + 4 * n2
                    for t in range(3):
                        nc.tensor.matmul(
                            ps1[:, 512 * n2:512 * n2 + 512],
                            w1s[hb:hb + 48, t, :],
                            stk3[hb:hb + 48, r0:r0 + 4, t + 1:t + 1 + W],
                            start=(t == 0), stop=(t == 2))
                dst = h1p.tile([128, HC], BF16, tag="h1")
                nc.scalar.activation(dst, ps1, AF.Relu, bias=b1s)
                return dst

            def l2(h1half):
                ps2 = ps2p.tile([128, HC], F32, tag="ps2")
                for m in range(2):
                    nc.tensor.matmul(
                        ps2[:, 512 * m:512 * m + 512], w2s,
                        h1half[:, 512 * m:512 * m + 512],
                        start=True, stop=True)
                return ps2

            def h2ev(k, h, ps2, mt):
                h2m = h2p.tile([128, HC], BF16, tag="h2m")
                if fuse_mask:
                    nc.vector.scalar_tensor_tensor(
                        h2m, ps2, 0.0, mt[:, k, HC * h:HC * h + HC],
                        op0=OP.max, op1=OP.mult)
                else:
                    nc.scalar.activation(h2m, ps2, AF.Relu, bias=b2s)
                    nc.vector.tensor_tensor(
                        h2m, h2m, mt[:, k, HC * h:HC * h + HC], OP.mult)
                return h2m

            def l3(i, h2m, ps3):
                cp = 32 * i
                for m in range(2):
                    nc.tensor.matmul(
                        ps3[cp:cp + 32, 512 * m:512 * m + 512], w3s,
                        h2m[:, 512 * m:512 * m + 512],
                        start=True, stop=True, tile_position=(0, cp))

            def xn_evac(k, h, ps3, X, xn3):
                xi_h = X[0:112,
                         16 * k + 1 + 8 * h:16 * k + 1 + 8 * h + 8,
                         2:2 + W]
                nc.vector.scalar_tensor_tensor(
                    xn3[0:112, 8 * h:8 * h + 8, :],
                    ps3[0:112, :].rearrange("p (r w) -> p r w", r=8),
                    b3s[0:112, :], xi_h, op0=OP.add, op1=OP.add)

            def epilogue(ep):
                """xnm = xn*alive (GPSIMD), Y = tanh(xnm) (ACT), seed for
                chunk 2; on the final step stream this chunk to DRAM."""
                Y, k, xn, at, final = ep
                xnm = xmp.tile([128, CC], BF16, tag="xnm")
                at3 = at[:, k, :].rearrange("p (r w) -> p r w", r=RCH)
                xn3 = xn.rearrange("p (r w) -> p r w", r=RCH)
                xnm3 = xnm.rearrange("p (r w) -> p r w", r=RCH)
                nc.gpsimd.tensor_tensor(
                    xnm3[0:112], xn3[0:112], at3[0:112], OP.mult)
                yi = Y[0:112, 16 * k + 1:16 * k + 1 + RCH, 2:2 + W]
                nc.scalar.activation(yi, xnm3[0:112], AF.Tanh)
                if k == 2:      # seed: image (32,32) = tile (33,34)
                    for b in range(NB):
                        nc.scalar.dma_start(
                            out=Y[32 * b + 3:32 * b + CH, 33:34, 34:35],
                            in_=seedd[:].to_broadcast([CH - 3, 1]))
                if final:
                    for b in range(NB):
                        nc.gpsimd.dma_start(
                            out=xout[b, :, 16 * k:16 * k + RCH, :],
                            in_=Y[32 * b:32 * b + CH,
                                  16 * k + 1:16 * k + 1 + RCH, 2:2 + W])

            def step_emit(X, Y, mt, carry, Xnext=None, final=False):
                """Emit one step.  carry = [stks(ch0), h0s(ch0)] produced by
                the previous step's iteration 7 (or the manual prologue).
                Returns the carry for the next step (or None)."""
                at = None
                for k in range(NCHUNK):
                    stksK, h0s = carry
                    xn = xnp.tile([128, CC], F32, tag="xn")
                    xn3 = xn.rearrange("p (r w) -> p r w", r=RCH)
                    # ---- B(k): h=0 through the MLP + trios h=1; L3 is
                    # skewed one image so PE never waits on the DVE evac ----
                    h1s = []
                    ps3a = ps3p.tile([128, HC], F32, tag="ps3")
                    lpend = None
                    for i in range(NB):
                        ps2 = l2(h0s[i])
                        h1s.append(trio(i, 1, stksK))
                        if lpend is not None:
                            l3(*lpend)
                        lpend = (i, h2ev(k, 0, ps2, mt), ps3a)
                    l3(*lpend)
                    xn_evac(k, 0, ps3a, X, xn3)
                    # prefetch stacks for the next merged-A target
                    if k < NCHUNK - 1:
                        stks_nxt = build_stks(k + 1, X)
                    elif Xnext is not None:
                        stks_nxt = build_stks(0, Xnext)
                    else:
                        stks_nxt = None
                    # ---- C(k) merged with A(next), same skew ----
                    ps3b = ps3p.tile([128, HC], F32, tag="ps3")
                    h0s_nxt = []
                    lpend = None
                    for i in range(NB):
                        ps2 = l2(h1s[i])
                        if stks_nxt is not None:
                            h0s_nxt.append(trio(i, 0, stks_nxt))
                        if lpend is not None:
                            l3(*lpend)
                        lpend = (i, h2ev(k, 1, ps2, mt), ps3b)
                    l3(*lpend)
                    xn_evac(k, 1, ps3b, X, xn3)
                    # ---- housekeeping (after the relu stream: ACT's tanh
                    # of the previous chunk must not delay this chunk) ----
                    if pend:
                        epilogue(pend.pop())
                    if k == 0:
                        at = alive_chain(X)
                    if k == 6 and mt_next:
                        mt_next[0] = mt_bcast(*mt_next[0])
                    pend.append((Y, k, xn, at, final))
                    carry = [stks_nxt, h0s_nxt]
                return carry if carry[0] is not None else None

            def prologue_A(X):
                stks0 = build_stks(0, X)
                return [stks0, [trio(i, 0, stks0) for i in range(NB)]]

            if loop_n is not None:
                with tc.For_i(0, loop_n, 2) as si:
                    mtA = mt_bcast(None, (si, 0))
                    mt_next.clear()
                    mt_next.append((None, (si, 1)))
                    carry = prologue_A(XA)
                    carry = step_emit(XA, XB, mtA, carry, Xnext=XB)
                    mtB = mt_next[0]
                    mt_next.clear()
                    step_emit(XB, XA, mtB, carry)
                    if pend:
                        epilogue(pend.pop())
                # timing variant: single writeout at the end
                for b in range(NB):
                    nc.gpsimd.dma_start(
                        out=xout[b, :, :, :],
                        in_=XA[32 * b:32 * b + CH, 1:H + 1, 2:2 + W])
            else:
                ping = [XA, XB]
                mt = mt_bcast(0)
                carry = prologue_A(XA)
                for si in range(steps):
                    X, Y = ping[si % 2], ping[(si + 1) % 2]
                    mt_next.clear()
                    if si + 1 < steps:
                        mt_next.append((si + 1,))
                    carry = step_emit(
                        X, Y, mt, carry,
                        Xnext=(Y if si + 1 < steps else None),
                        final=(si == steps - 1))
                    if mt_next:
                        mt = mt_next[0]
                if pend:
                    epilogue(pend.pop())

    nc.compile()
    return nc


def _prep_weights(w1, b1, w2, b2, w3, b3):
    import ml_dtypes
    bf = ml_dtypes.bfloat16
    w1 = np.asarray(w1, np.float32)
    W1x, W1gx, W1gy, W1lap = (w1[:, 0:16], w1[:, 16:32],
                              w1[:, 32:48], w1[:, 48:64])
    A = (KX[:, :, None, None] * W1gx[None, None] +
         KY[:, :, None, None] * W1gy[None, None] +
         KL[:, :, None, None] * W1lap[None, None])
    A = A.astype(np.float32).copy()
    A[1, 1] += W1x
    blk = A.transpose(0, 3, 1, 2).reshape(48, 3, HID)
    w1t = np.zeros((128, 3, HID), np.float32)
    w1t[0:48] = blk
    w1t[64:112] = blk
    w2t = np.ascontiguousarray(np.asarray(w2, np.float32).T)
    w3t = np.zeros((HID, 2 * CH), np.float32)
    w3t[:, 0:CH] = 0.05 * np.asarray(w3, np.float32).T
    b1v = np.asarray(b1, np.float32).reshape(HID, 1)
    b2v = np.asarray(b2, np.float32).reshape(HID, 1)
    b3v = np.zeros((128, 1), np.float32)
    for b in range(NB):
        b3v[32 * b:32 * b + CH, 0] = 0.05 * np.asarray(b3, np.float32)
    return (w1t.astype(bf), w2t.astype(bf), w3t.astype(bf), b1v, b2v, b3v)


LAST_EXEC_NS = None
LAST_PROFILE_JSON = None
LAST_TRACE_PATH = None


def kernel(x, rand, w1, b1, w2, b2, w3, b3, steps=STEPS):
    import ml_dtypes
    from concourse import mybir
    from concourse.bass_utils import run_bass_kernel_spmd

    import os
    fuse_mask = bool(np.all(np.asarray(b2) == 0.0))
    key = (steps, fuse_mask)
    if key not in _BUILD_CACHE:
        _BUILD_CACHE[key] = _build(steps, fuse_mask=fuse_mask)
    nc = _BUILD_CACHE[key]

    f8 = mybir.dt.np(mybir.dt.float8e4)
    x = np.asarray(x, np.float32)
    rand = np.asarray(rand, np.float32)
    w1t, w2t, w3t, b1v, b2v, b3v = _prep_weights(w1, b1, w2, b2, w3, b3)
    m8c = (rand < 0.5).astype(np.float32).reshape(STEPS * NCHUNK, CC)
    if isinstance(steps, tuple):
        reps = -(-steps[1] * NCHUNK // m8c.shape[0])
        m8c = np.tile(m8c, (reps, 1))[:steps[1] * NCHUNK]
    m8c = m8c.astype(f8)

    in_maps = []
    for c in range(NCORES):
        shard = x[c * NB:(c + 1) * NB]                 # [4, 16, H, W]
        xp = np.zeros((NB, CH, PH, PW), np.float32)
        xp[:, :, 1:H + 1, 2:2 + W] = shard
        xp = xp.astype(ml_dtypes.bfloat16)
        in_maps.append({
            "xin": xp, "m8c": m8c, "w1t": w1t, "w2t": w2t,
            "w3t": w3t, "b1d": b1v, "b2d": b2v, "b3d": b3v,
            "seedd": np.array([np.tanh(1.0)], ml_dtypes.bfloat16),
        })

    run_kwargs = {}
    if os.environ.get("NCA_TRACE"):
        run_kwargs["trace"] = True
        td = os.environ.get("NCA_TRACE_DIR")
        if td:
            os.makedirs(td, exist_ok=True)
            run_kwargs["tmpdir"] = td
    res = run_bass_kernel_spmd(nc, in_maps, core_ids=list(range(NCORES)),
                               **run_kwargs)
    if os.environ.get("NCA_TRACE"):
        global LAST_EXEC_NS, LAST_PROFILE_JSON, LAST_TRACE_PATH
        LAST_EXEC_NS = res.exec_time_ns
        LAST_PROFILE_JSON = res.profile_json
        if res.instructions_and_trace is not None:
            LAST_TRACE_PATH = res.instructions_and_trace[1]
    out = np.concatenate([r["xout"] for r in res.results], axis=0)
    return out.astype(np.float32)


# revision 3
# speedup vs baseline: 2.0470x; 1.0301x over previous
"""Trainium2 Bass kernel for nn_EnhancedStableNCA (v3).

Strategy (per core, data-parallel over batch: 4 images/core on 8 cores):
- Ping-pong full-state tiles XA/XB [128, 130, 132] bf16 (partition 32b+c,
  tile row = image row + 1, tile col = image col + 2; zero borders).  Each
  step reads X and writes Y: no halo refresh, no intra-step RAW hazards.
- The three 3x3 depthwise convs fold into the layer-1 matmul: per chunk of
  16 image rows a y-stack tile [128,16,132] holds 3 dy-shifted copies of
  2 images (partition 64i+16d+c, one 3-dim DMA per dy); 3 dx-tap matmuls
  (K=48, M=1024) accumulate in PSUM.
- M=1024 matmuls throughout (each writes a [128,1024] f32 PSUM = 2 banks).
- h1 = relu(ps1+b1) on ACT.  h2m = relu(ps2)*mask via one DVE stt (b2==0).
- L3 col-packed (4 images at PE col positions 32i); xn = (ps3+b3)+x via
  DVE stt; xnm = xn*alive on GPSIMD; Y = tanh(xnm) on ACT (valid since
  alive is 0/1: alive*tanh(z) == tanh(alive*z)).
- alive: one whole-step alpha gather DMA (ACT queue, right after the last
  tanh of the previous step) into ac [32=8b+k, 18, 132]; threshold+boxsum
  on DVE -> av [32, 2048]; one broadcast DMA -> at [128, 8, 2048] fp8.
  rand mask mt is one broadcast DMA per step, prefetched a chunk early.
- Epilogue of chunk g runs software-pipelined inside chunk g+1 so ACT's
  relu stream never stalls behind tanh.
"""

import numpy as np

CH = 16
HID = 128
B, H, W = 32, 128, 128
STEPS = 8
NCORES = 8
NB = B // NCORES          # images per core
NCHUNK = 8                # row chunks per image
RCH = H // NCHUNK         # rows per chunk (16)
PW = W + 4                # padded row width (132); interior cols 2..129
PH = H + 2                # padded rows (130); interior rows 1..128
CC = RCH * W              # chunk pixels per image (2048)
HC = CC // 2              # half-chunk pixels (1024)

KX = np.array([[-1., 0., 1.], [-2., 0., 2.], [-1., 0., 1.]], np.float32) / 8.0
KY = np.array([[-1., -2., -1.], [0., 0., 0.], [1., 2., 1.]], np.float32) / 8.0
KL = np.array([[0.05, 0.2, 0.05], [0.2, -1.0, 0.2], [0.05, 0.2, 0.05]], np.float32)

_BUILD_CACHE = {}


def _build(steps, fuse_mask=True):
    import os
    import concourse.bacc as bacc
    import concourse.bass as bass
    import concourse.tile as tile
    from concourse import mybir

    _dbg = os.environ.get("NCA_DBG", "")

    F32 = mybir.dt.float32
    BF16 = mybir.dt.bfloat16
    FP8 = mybir.dt.float8e4
    AF = mybir.ActivationFunctionType
    OP = mybir.AluOpType
    AP = bass.AP
    loop_n = None
    if isinstance(steps, tuple):          # ("loop", N) timing variant
        loop_n = steps[1]
        assert loop_n % 2 == 0, "loop variant needs even step count"
    msteps = loop_n if loop_n is not None else steps

    nc = bacc.Bacc("TRN2", target_bir_lowering=False, debug=False)

    # ---- DRAM I/O ----
    xin = nc.dram_tensor("xin", [NB, CH, PH, PW], BF16, kind="ExternalInput")
    m8c = nc.dram_tensor("m8c", [msteps * NCHUNK, CC], FP8, kind="ExternalInput")
    w1t = nc.dram_tensor("w1t", [128, 3, HID], BF16, kind="ExternalInput")
    w2t = nc.dram_tensor("w2t", [HID, HID], BF16, kind="ExternalInput")
    w3t = nc.dram_tensor("w3t", [HID, 2 * CH], BF16, kind="ExternalInput")
    b1d = nc.dram_tensor("b1d", [HID, 1], F32, kind="ExternalInput")
    b2d = nc.dram_tensor("b2d", [HID, 1], F32, kind="ExternalInput")
    b3d = nc.dram_tensor("b3d", [128, 1], F32, kind="ExternalInput")
    seedd = nc.dram_tensor("seedd", [1], BF16, kind="ExternalInput")
    xout = nc.dram_tensor("xout", [NB, CH, H, W], F32, kind="ExternalOutput")

    XEL = PH * PW            # state tile free elems per partition (17160)
    ACEL = (RCH + 2) * PW    # ac tile free elems (2376)
    SEEDV = float(np.tanh(1.0))

    with tile.TileContext(nc) as tc:
        with tc.tile_pool(name="state", bufs=1) as state, \
             tc.tile_pool(name="stkp", bufs=4) as stkp, \
             tc.tile_pool(name="h1p", bufs=10) as h1p, \
             tc.tile_pool(name="h2p", bufs=3) as h2p, \
             tc.tile_pool(name="xnp", bufs=2) as xnp, \
             tc.tile_pool(name="xmp", bufs=2) as xmp, \
             tc.tile_pool(name="mtp", bufs=2) as mtp, \
             tc.tile_pool(name="atp", bufs=1) as atp, \
             tc.tile_pool(name="acp", bufs=1) as acp, \
             tc.tile_pool(name="avp", bufs=2) as avp, \
             tc.tile_pool(name="mpp", bufs=1) as mpp, \
             tc.tile_pool(name="dram", bufs=2, space="DRAM") as dramp, \
             tc.tile_pool(name="ps1p", bufs=2, space="PSUM") as ps1p, \
             tc.tile_pool(name="ps2p", bufs=1, space="PSUM") as ps2p, \
             tc.tile_pool(name="ps3p", bufs=1, space="PSUM") as ps3p:

            # ---- persistent state ----
            XA = state.tile([128, PH, PW], BF16, tag="XA", name="XA")
            XB = state.tile([128, PH, PW], BF16, tag="XB", name="XB")
            w1s = state.tile([128, 3, HID], BF16, tag="w1s")
            w2s = state.tile([HID, HID], BF16, tag="w2s")
            w3s = state.tile([HID, 2 * CH], BF16, tag="w3s")
            b1s = state.tile([HID, 1], F32, tag="b1s")
            b2s = state.tile([HID, 1], F32, tag="b2s")
            b3s = state.tile([128, 1], F32, tag="b3s")

            nc.sync.dma_start(out=w1s, in_=w1t[:])
            nc.sync.dma_start(out=w2s, in_=w2t[:])
            nc.sync.dma_start(out=w3s, in_=w3t[:])
            nc.sync.dma_start(out=b1s, in_=b1d[:])
            nc.sync.dma_start(out=b2s, in_=b2d[:])
            nc.sync.dma_start(out=b3s, in_=b3d[:])

            nc.vector.memset(XA, 0.0)
            nc.vector.memset(XB, 0.0)
            for b in range(NB):
                nc.sync.dma_start(out=XA[32 * b:32 * b + CH, :, :],
                                  in_=xin[b, :, :, :])

            def mt_bcast(si, si_pair=None):
                """rand mask -> mt [128, 8, 2048] fp8, 1 DMA (ACT queue)."""
                mt = mtp.tile([128, NCHUNK, CC], FP8, tag="mt")
                if "nobc" in _dbg:
                    nc.vector.memset(mt, 1.0)
                    return mt
                if si_pair is None:
                    msrc = m8c[si * NCHUNK:(si + 1) * NCHUNK, :]
                else:
                    li, j = si_pair
                    msrc = m8c[bass.ds(NCHUNK * li + NCHUNK * j,
                                       NCHUNK), :]
                nc.scalar.dma_start(
                    out=mt,
                    in_=msrc.unsqueeze(0).to_broadcast([128, NCHUNK, CC]))
                return mt

            def alive_chain(X):
                """gather alpha from X (ACT queue DMA), threshold+boxsum on
                DVE, broadcast to at [128, 8, 2048] (DVE queue DMA)."""
                at = atp.tile([128, NCHUNK, CC], FP8, tag="at")
                if "nobc" in _dbg:
                    nc.vector.memset(at, 1.0)
                    return at
                ac = acp.tile([32, RCH + 2, PW], BF16, tag="ac")
                for b in range(NB):
                    nc.scalar.dma_start(
                        out=ac[NCHUNK * b:NCHUNK * b + NCHUNK, :, :],
                        in_=AP(X[:].tensor, (32 * b + 3) * XEL,
                               [[XEL, 1], [RCH * PW, NCHUNK], [1, ACEL]]))
                ab = mpp.tile([32, RCH + 2, PW], BF16, tag="ab")
                nc.vector.tensor_single_scalar(ab, ac, 0.01, OP.is_gt)
                d1 = mpp.tile([32, RCH + 2, W], BF16, tag="d1")
                nc.vector.tensor_tensor(
                    d1, ab[:, :, 1:W + 1], ab[:, :, 3:W + 3], OP.add)
                nc.vector.tensor_tensor(d1, d1, ab[:, :, 2:W + 2], OP.add)
                d2 = mpp.tile([32, RCH, W], BF16, tag="d2")
                nc.vector.tensor_tensor(
                    d2, d1[:, 0:RCH], d1[:, 2:RCH + 2], OP.add)
                nc.vector.tensor_tensor(d2, d2, d1[:, 1:RCH + 1], OP.add)
                av = avp.tile([32, CC], FP8, tag="av")
                nc.vector.tensor_single_scalar(
                    av, d2.rearrange("p r w -> p (r w)"), 0.5, OP.is_gt)
                # broadcast av -> at via DRAM bounce (partition replication
                # needs a linear source): at[32b+c, k, pix] = av[8b+k, pix]
                avd = dramp.tile([32, CC], FP8, tag="avd", name="avd")
                nc.gpsimd.dma_start(out=avd, in_=av)
                for b in range(NB):
                    nc.gpsimd.dma_start(
                        out=at[32 * b:32 * b + 32, :, :],
                        in_=avd[NCHUNK * b:NCHUNK * b + NCHUNK, :]
                            .rearrange("k c -> (k c)").unsqueeze(0)
                            .to_broadcast([32, NCHUNK * CC])
                            .rearrange("c (k p) -> c k p", k=NCHUNK))
                return at

            def epilogue(ep):
                """Pipelined chunk epilogue: xnm = xn*alive (GPSIMD), then
                Y = tanh(xnm) (ACT) + seed DMAs for chunk 2; on the final
                step also stream this chunk's rows to DRAM."""
                Y, k, xn, at, final = ep
                xnm = xmp.tile([128, CC], BF16, tag="xnm")
                at3 = at[:, k, :].rearrange("p (r w) -> p r w", r=RCH)
                xn3 = xn.rearrange("p (r w) -> p r w", r=RCH)
                if "nogps" in _dbg:
                    nc.vector.tensor_tensor(xnm[0:112], xn3[0:112].rearrange(
                        "p r w -> p (r w)"), at[0:112, k, :], OP.mult)
                else:
                    nc.gpsimd.tensor_tensor(
                        xnm.rearrange("p (r w) -> p r w", r=RCH)[0:112],
                        xn3[0:112], at3[0:112], OP.mult)
                yi = Y[0:112, 16 * k + 1:16 * k + 1 + RCH, 2:2 + W]
                nc.scalar.activation(
                    yi, xnm.rearrange("p (r w) -> p r w", r=RCH)[0:112],
                    AF.Tanh)
                if k == 2:      # seed: image (32,32) = tile (33,34)
                    for b in range(NB):
                        nc.scalar.dma_start(
                            out=Y[32 * b + 3:32 * b + CH, 33:34, 34:35],
                            in_=seedd[:].to_broadcast([CH - 3, 1]))
                if final:
                    for b in range(NB):
                        nc.gpsimd.dma_start(
                            out=xout[b, :, 16 * k:16 * k + RCH, :],
                            in_=Y[32 * b:32 * b + CH,
                                  16 * k + 1:16 * k + 1 + RCH, 2:2 + W])

            # ---------------- software-pipelined emission ----------------
            # Iteration k emits: B(k) [L2/h2ev/L3 h=0 + trio h=1],
            # xn_evac(0,k), stk-prefetch(k+1), epilogue(k-1), C(k)+A(k+1)
            # merged per image [L2(i,1); trio(i,0,k+1); h2ev; L3(i,1)],
            # xn_evac(1,k).  A(k+1) trios cover h2ev latency so PE never
            # stalls between L2 and L3; the PE stream also crosses step
            # boundaries seamlessly (A of chunk 0 next step is emitted in
            # iteration 7 of this step).
            pend = []     # pending epilogue args
            mt_next = []  # prefetched mask tile
            PWS = PW + 2               # padded stk row (no AP merges)
            SPIT = RCH * PWS           # stk partition pitch

            def build_stks(k, X):
                stks = []
                for j in range(2):
                    stk = stkp.tile([128, RCH, PWS], BF16, tag="stk")
                    for i in range(2):
                        for d in range(3):
                            nc.sync.dma_start(
                                out=AP(stk[:].tensor,
                                       (64 * i + 16 * d) * SPIT,
                                       [[SPIT, CH], [1, RCH * PW]]),
                                in_=AP(X[:].tensor,
                                       (64 * j + 32 * i) * XEL
                                       + (16 * k + d) * PW,
                                       [[XEL, CH], [1, RCH * PW]]))
                    stks.append(stk)
                return stks

            def trio(i, h, stks):
                """L1 for image i, half h: 6 matmuls + relu evac.
                Returns the [128, 1024] bf16 h1-half tile."""
                stkf = stks[i // 2].rearrange("p r w -> p (r w)")
                stk3 = AP(stkf.tensor, 0,
                          [[RCH * PWS, 128], [PW, RCH], [1, PW]])
                hb = 64 * (i % 2)
                ps1 = ps1p.tile([128, HC], F32, tag="ps1")
                for n2 in range(2):
                    r0 = 8 * h + 4 * n2
                    for t in range(3):
                        nc.tensor.matmul(
                            ps1[:, 512 * n2:512 * n2 + 512],
                            w1s[hb:hb + 48, t, :],
                            stk3[hb:hb + 48, r0:r0 + 4, t + 1:t + 1 + W],
                            start=(t == 0), stop=(t == 2))
                dst = h1p.tile([128, HC], BF16, tag="h1")
                nc.scalar.activation(dst, ps1, AF.Relu, bias=b1s)
                return dst

            def l2(h1half):
                ps2 = ps2p.tile([128, HC], F32, tag="ps2")
                for m in range(2):
                    nc.tensor.matmul(
                        ps2[:, 512 * m:512 * m + 512], w2s,
                        h1half[:, 512 * m:512 * m + 512],
                        start=True, stop=True)
                return ps2

            def h2ev(k, h, ps2, mt):
                h2m = h2p.tile([128, HC], BF16, tag="h2m")
                if fuse_mask:
                    nc.vector.scalar_tensor_tensor(
                        h2m, ps2, 0.0, mt[:, k, HC * h:HC * h + HC],
                        op0=OP.max, op1=OP.mult)
                else:
                    nc.scalar.activation(h2m, ps2, AF.Relu, bias=b2s)
                    nc.vector.tensor_tensor(
                        h2m, h2m, mt[:, k, HC * h:HC * h + HC], OP.mult)
                return h2m

            def l3(i, h2m, ps3):
                cp = 32 * i
                for m in range(2):
                    nc.tensor.matmul(
                        ps3[cp:cp + 32, 512 * m:512 * m + 512], w3s,
                        h2m[:, 512 * m:512 * m + 512],
                        start=True, stop=True, tile_position=(0, cp))

            def xn_evac(k, h, ps3, X, xn3):
                xi_h = X[0:112,
                         16 * k + 1 + 8 * h:16 * k + 1 + 8 * h + 8,
                         2:2 + W]
                nc.vector.scalar_tensor_tensor(
                    xn3[0:112, 8 * h:8 * h + 8, :],
                    ps3[0:112, :].rearrange("p (r w) -> p r w", r=8),
                    b3s[0:112, :], xi_h, op0=OP.add, op1=OP.add)

            def epilogue(ep):
                """xnm = xn*alive (GPSIMD), Y = tanh(xnm) (ACT), seed for
                chunk 2; on the final step stream this chunk to DRAM."""
                Y, k, xn, at, final = ep
                xnm = xmp.tile([128, CC], BF16, tag="xnm")
                at3 = at[:, k, :].rearrange("p (r w) -> p r w", r=RCH)
                xn3 = xn.rearrange("p (r w) -> p r w", r=RCH)
                xnm3 = xnm.rearrange("p (r w) -> p r w", r=RCH)
                nc.gpsimd.tensor_tensor(
                    xnm3[0:112], xn3[0:112], at3[0:112], OP.mult)
                yi = Y[0:112, 16 * k + 1:16 * k + 1 + RCH, 2:2 + W]
                nc.scalar.activation(yi, xnm3[0:112], AF.Tanh)
                if k == 2:      # seed: image (32,32) = tile (33,34)
                    for b in range(NB):
                        nc.scalar.dma_start(
                            out=Y[32 * b + 3:32 * b + CH, 33:34, 34:35],
                            in_=seedd[:].to_broadcast([CH - 3, 1]))
                if final:
                    for b in range(NB):
                        nc.gpsimd.dma_start(
                            out=xout[b, :, 16 * k:16 * k + RCH, :],
                            in_=Y[32 * b:32 * b + CH,
                                  16 * k + 1:16 * k + 1 + RCH, 2:2 + W])

            def step_emit(X, Y, mt, carry, Xnext=None, final=False):
                """Emit one step.  carry = [stks(ch0), h0s(ch0)] produced by
                the previous step's iteration 7 (or the manual prologue).
                Returns the carry for the next step (or None)."""
                at = None
                for k in range(NCHUNK):
                    stksK, h0s = carry
                    xn = xnp.tile([128, CC], F32, tag="xn")
                    xn3 = xn.rearrange("p (r w) -> p r w", r=RCH)
                    # ---- B(k): h=0 through the MLP + trios h=1; L3 is
                    # skewed one image so PE never waits on the DVE evac ----
                    h1s = []
                    ps3a = ps3p.tile([128, HC], F32, tag="ps3")
                    lpend = None
                    for i in range(NB):
                        ps2 = l2(h0s[i])
                        h1s.append(trio(i, 1, stksK))
                        if lpend is not None:
                            l3(*lpend)
                        lpend = (i, h2ev(k, 0, ps2, mt), ps3a)
                    l3(*lpend)
                    xn_evac(k, 0, ps3a, X, xn3)
                    # prefetch stacks for the next merged-A target
                    if k < NCHUNK - 1:
                        stks_nxt = build_stks(k + 1, X)
                    elif Xnext is not None:
                        stks_nxt = build_stks(0, Xnext)
                    else:
                        stks_nxt = None
                    # ---- C(k) merged with A(next), same skew ----
                    ps3b = ps3p.tile([128, HC], F32, tag="ps3")
                    h0s_nxt = []
                    lpend = None
                    for i in range(NB):
                        ps2 = l2(h1s[i])
                        if stks_nxt is not None:
                            h0s_nxt.append(trio(i, 0, stks_nxt))
                        if lpend is not None:
                            l3(*lpend)
                        lpend = (i, h2ev(k, 1, ps2, mt), ps3b)
                    l3(*lpend)
                    xn_evac(k, 1, ps3b, X, xn3)
                    # ---- housekeeping (after the relu stream: ACT's tanh
                    # of the previous chunk must not delay this chunk) ----
                    if pend:
                        epilogue(pend.pop())
                    if k == 0:
                        at = alive_chain(X)
                    if k == 6 and mt_next:
                        mt_next[0] = mt_bcast(*mt_next[0])
                    pend.append((Y, k, xn, at, final))
                    carry = [stks_nxt, h0s_nxt]
                return carry if carry[0] is not None else None

            def prologue_A(X):
                stks0 = build_stks(0, X)
                return [stks0, [trio(i, 0, stks0) for i in range(NB)]]

            if loop_n is not None:
                with tc.For_i(0, loop_n, 2, staggered_reset=True) as si:
                    mtA = mt_bcast(None, (si, 0))
                    mt_next.clear()
                    mt_next.append((None, (si, 1)))
                    carry = prologue_A(XA)
                    carry = step_emit(XA, XB, mtA, carry, Xnext=XB)
                    mtB = mt_next[0]
                    mt_next.clear()
                    step_emit(XB, XA, mtB, carry)
                    if pend:
                        epilogue(pend.pop())
                # timing variant: single writeout at the end
                for b in range(NB):
                    nc.gpsimd.dma_start(
                        out=xout[b, :, :, :],
                        in_=XA[32 * b:32 * b + CH, 1:H + 1, 2:2 + W])
            else:
                ping = [XA, XB]
                mt = mt_bcast(0)
                carry = prologue_A(XA)
                for si in range(steps):
                    X, Y = ping[si % 2], ping[(si + 1) % 2]
                    mt_next.clear()
                    if si + 1 < steps:
                        mt_next.append((si + 1,))
                    carry = step_emit(
                        X, Y, mt, carry,
                        Xnext=(Y if si + 1 < steps else None),
                        final=(si == steps - 1))
                    if mt_next:
                        mt = mt_next[0]
                if pend:
                    epilogue(pend.pop())

    nc.compile()
    return nc


def _prep_weights(w1, b1, w2, b2, w3, b3):
    import ml_dtypes
    bf = ml_dtypes.bfloat16
    w1 = np.asarray(w1, np.float32)
    W1x, W1gx, W1gy, W1lap = (w1[:, 0:16], w1[:, 16:32],
                              w1[:, 32:48], w1[:, 48:64])
    A = (KX[:, :, None, None] * W1gx[None, None] +
         KY[:, :, None, None] * W1gy[None, None] +
         KL[:, :, None, None] * W1lap[None, None])
    A = A.astype(np.float32).copy()
    A[1, 1] += W1x
    blk = A.transpose(0, 3, 1, 2).reshape(48, 3, HID)
    w1t = np.zeros((128, 3, HID), np.float32)
    w1t[0:48] = blk
    w1t[64:112] = blk
    w2t = np.ascontiguousarray(np.asarray(w2, np.float32).T)
    w3t = np.zeros((HID, 2 * CH), np.float32)
    w3t[:, 0:CH] = 0.05 * np.asarray(w3, np.float32).T
    b1v = np.asarray(b1, np.float32).reshape(HID, 1)
    b2v = np.asarray(b2, np.float32).reshape(HID, 1)
    b3v = np.zeros((128, 1), np.float32)
    for b in range(NB):
        b3v[32 * b:32 * b + CH, 0] = 0.05 * np.asarray(b3, np.float32)
    return (w1t.astype(bf), w2t.astype(bf), w3t.astype(bf), b1v, b2v, b3v)


LAST_EXEC_NS = None
LAST_PROFILE_JSON = None
LAST_TRACE_PATH = None


def kernel(x, rand, w1, b1, w2, b2, w3, b3, steps=STEPS):
    import ml_dtypes
    from concourse import mybir
    from concourse.bass_utils import run_bass_kernel_spmd

    import os
    fuse_mask = bool(np.all(np.asarray(b2) == 0.0))
    key = (steps, fuse_mask)
    if key not in _BUILD_CACHE:
        _BUILD_CACHE[key] = _build(steps, fuse_mask=fuse_mask)
    nc = _BUILD_CACHE[key]

    f8 = mybir.dt.np(mybir.dt.float8e4)
    x = np.asarray(x, np.float32)
    rand = np.asarray(rand, np.float32)
    w1t, w2t, w3t, b1v, b2v, b3v = _prep_weights(w1, b1, w2, b2, w3, b3)
    m8c = (rand < 0.5).astype(np.float32).reshape(STEPS * NCHUNK, CC)
    if isinstance(steps, tuple):
        reps = -(-steps[1] * NCHUNK // m8c.shape[0])
        m8c = np.tile(m8c, (reps, 1))[:steps[1] * NCHUNK]
    m8c = m8c.astype(f8)

    in_maps = []
    for c in range(NCORES):
        shard = x[c * NB:(c + 1) * NB]                 # [4, 16, H, W]
        xp = np.zeros((NB, CH, PH, PW), np.float32)
        xp[:, :, 1:H + 1, 2:2 + W] = shard
        xp = xp.astype(ml_dtypes.bfloat16)
        in_maps.append({
            "xin": xp, "m8c": m8c, "w1t": w1t, "w2t": w2t,
            "w3t": w3t, "b1d": b1v, "b2d": b2v, "b3d": b3v,
            "seedd": np.array([np.tanh(1.0)], ml_dtypes.bfloat16),
        })

    run_kwargs = {}
    if os.environ.get("NCA_TRACE"):
        run_kwargs["trace"] = True
        td = os.environ.get("NCA_TRACE_DIR")
        if td:
            os.makedirs(td, exist_ok=True)
            run_kwargs["tmpdir"] = td
    res = run_bass_kernel_spmd(nc, in_maps, core_ids=list(range(NCORES)),
                               **run_kwargs)
    if os.environ.get("NCA_TRACE"):
        global LAST_EXEC_NS, LAST_PROFILE_JSON, LAST_TRACE_PATH
        LAST_EXEC_NS = res.exec_time_ns
        LAST_PROFILE_JSON = res.profile_json
        if res.instructions_and_trace is not None:
            LAST_TRACE_PATH = res.instructions_and_trace[1]
    out = np.concatenate([r["xout"] for r in res.results], axis=0)
    return out.astype(np.float32)


# revision 4
# speedup vs baseline: 2.1818x; 1.0658x over previous
"""Trainium2 Bass kernel for nn_EnhancedStableNCA (v3).

Strategy (per core, data-parallel over batch: 4 images/core on 8 cores):
- Ping-pong full-state tiles XA/XB [128, 130, 132] bf16 (partition 32b+c,
  tile row = image row + 1, tile col = image col + 2; zero borders).  Each
  step reads X and writes Y: no halo refresh, no intra-step RAW hazards.
- The three 3x3 depthwise convs fold into the layer-1 matmul: per chunk of
  16 image rows a y-stack tile [128,16,132] holds 3 dy-shifted copies of
  2 images (partition 64i+16d+c, one 3-dim DMA per dy); 3 dx-tap matmuls
  (K=48, M=1024) accumulate in PSUM.
- M=1024 matmuls throughout (each writes a [128,1024] f32 PSUM = 2 banks).
- h1 = relu(ps1+b1) on ACT.  h2m = relu(ps2)*mask via one DVE stt (b2==0).
- L3 col-packed (4 images at PE col positions 32i); xn = (ps3+b3)+x via
  DVE stt; xnm = xn*alive on GPSIMD; Y = tanh(xnm) on ACT (valid since
  alive is 0/1: alive*tanh(z) == tanh(alive*z)).
- alive: one whole-step alpha gather DMA (ACT queue, right after the last
  tanh of the previous step) into ac [32=8b+k, 18, 132]; threshold+boxsum
  on DVE -> av [32, 2048]; one broadcast DMA -> at [128, 8, 2048] fp8.
  rand mask mt is one broadcast DMA per step, prefetched a chunk early.
- Epilogue of chunk g runs software-pipelined inside chunk g+1 so ACT's
  relu stream never stalls behind tanh.
"""

import numpy as np

CH = 16
HID = 128
B, H, W = 32, 128, 128
STEPS = 8
NCORES = 8
NB = B // NCORES          # images per core
NCHUNK = 8                # row chunks per image
RCH = H // NCHUNK         # rows per chunk (16)
PW = W + 4                # padded row width (132); interior cols 2..129
PH = H + 2                # padded rows (130); interior rows 1..128
CC = RCH * W              # chunk pixels per image (2048)
HC = CC // 2              # half-chunk pixels (1024)

KX = np.array([[-1., 0., 1.], [-2., 0., 2.], [-1., 0., 1.]], np.float32) / 8.0
KY = np.array([[-1., -2., -1.], [0., 0., 0.], [1., 2., 1.]], np.float32) / 8.0
KL = np.array([[0.05, 0.2, 0.05], [0.2, -1.0, 0.2], [0.05, 0.2, 0.05]], np.float32)

_BUILD_CACHE = {}


def _build(steps, fuse_mask=True):
    import os
    import concourse.bacc as bacc
    import concourse.bass as bass
    import concourse.tile as tile
    from concourse import mybir

    _dbg = os.environ.get("NCA_DBG", "")

    F32 = mybir.dt.float32
    BF16 = mybir.dt.bfloat16
    FP8 = mybir.dt.float8e4
    AF = mybir.ActivationFunctionType
    OP = mybir.AluOpType
    AP = bass.AP
    loop_n = None
    if isinstance(steps, tuple):          # ("loop", N) timing variant
        loop_n = steps[1]
        assert loop_n % 2 == 0, "loop variant needs even step count"
    msteps = loop_n if loop_n is not None else steps

    nc = bacc.Bacc("TRN2", target_bir_lowering=False, debug=False)

    # ---- DRAM I/O ----
    xin = nc.dram_tensor("xin", [NB, CH, PH, PW], BF16, kind="ExternalInput")
    m8c = nc.dram_tensor("m8c", [msteps * NCHUNK, CC], FP8, kind="ExternalInput")
    w1t = nc.dram_tensor("w1t", [128, 3, HID], BF16, kind="ExternalInput")
    w2t = nc.dram_tensor("w2t", [HID, HID], BF16, kind="ExternalInput")
    w3t = nc.dram_tensor("w3t", [HID, 2 * CH], BF16, kind="ExternalInput")
    b1d = nc.dram_tensor("b1d", [HID, 1], F32, kind="ExternalInput")
    b2d = nc.dram_tensor("b2d", [HID, 1], F32, kind="ExternalInput")
    b3d = nc.dram_tensor("b3d", [128, 1], F32, kind="ExternalInput")
    seedd = nc.dram_tensor("seedd", [1], BF16, kind="ExternalInput")
    xout = nc.dram_tensor("xout", [NB, CH, H, W], F32, kind="ExternalOutput")

    XEL = PH * PW            # state tile free elems per partition (17160)
    ACEL = (RCH + 2) * PW    # ac tile free elems (2376)
    SEEDV = float(np.tanh(1.0))

    with tile.TileContext(nc) as tc:
        with tc.tile_pool(name="state", bufs=1) as state, \
             tc.tile_pool(name="stkp", bufs=4) as stkp, \
             tc.tile_pool(name="h1p", bufs=10) as h1p, \
             tc.tile_pool(name="h2p", bufs=3) as h2p, \
             tc.tile_pool(name="xnp", bufs=2) as xnp, \
             tc.tile_pool(name="xmp", bufs=2) as xmp, \
             tc.tile_pool(name="mtp", bufs=2) as mtp, \
             tc.tile_pool(name="atp", bufs=1) as atp, \
             tc.tile_pool(name="acp", bufs=1) as acp, \
             tc.tile_pool(name="avp", bufs=2) as avp, \
             tc.tile_pool(name="mpp", bufs=1) as mpp, \
             tc.tile_pool(name="dram", bufs=2, space="DRAM") as dramp, \
             tc.tile_pool(name="ps1p", bufs=2, space="PSUM") as ps1p, \
             tc.tile_pool(name="ps2p", bufs=1, space="PSUM") as ps2p, \
             tc.tile_pool(name="ps3p", bufs=1, space="PSUM") as ps3p:

            # ---- persistent state ----
            XA = state.tile([128, PH, PW], BF16, tag="XA", name="XA")
            XB = state.tile([128, PH, PW], BF16, tag="XB", name="XB")
            w1s = state.tile([128, 3, HID], BF16, tag="w1s")
            w2s = state.tile([HID, HID], BF16, tag="w2s")
            w3s = state.tile([HID, 2 * CH], BF16, tag="w3s")
            b1s = state.tile([HID, 1], F32, tag="b1s")
            b2s = state.tile([HID, 1], F32, tag="b2s")
            b3s = state.tile([128, 1], F32, tag="b3s")

            nc.sync.dma_start(out=w1s, in_=w1t[:])
            nc.sync.dma_start(out=w2s, in_=w2t[:])
            nc.sync.dma_start(out=w3s, in_=w3t[:])
            nc.sync.dma_start(out=b1s, in_=b1d[:])
            nc.sync.dma_start(out=b2s, in_=b2d[:])
            nc.sync.dma_start(out=b3s, in_=b3d[:])

            nc.vector.memset(XA, 0.0)
            nc.vector.memset(XB, 0.0)
            for b in range(NB):
                nc.sync.dma_start(out=XA[32 * b:32 * b + CH, :, :],
                                  in_=xin[b, :, :, :])

            def mt_bcast(si, si_pair=None):
                """rand mask -> mt [128, 8, 2048] fp8, 1 DMA (ACT queue)."""
                mt = mtp.tile([128, NCHUNK, CC], FP8, tag="mt")
                if "nobc" in _dbg:
                    nc.vector.memset(mt, 1.0)
                    return mt
                if si_pair is None:
                    msrc = m8c[si * NCHUNK:(si + 1) * NCHUNK, :]
                else:
                    li, j = si_pair
                    msrc = m8c[bass.ds(NCHUNK * li + NCHUNK * j,
                                       NCHUNK), :]
                nc.scalar.dma_start(
                    out=mt,
                    in_=msrc.unsqueeze(0).to_broadcast([128, NCHUNK, CC]))
                return mt

            def alive_chain(X):
                """gather alpha from X (ACT queue DMA), threshold+boxsum on
                DVE, broadcast to at [128, 8, 2048] (DVE queue DMA)."""
                at = atp.tile([128, NCHUNK, CC], FP8, tag="at")
                if "nobc" in _dbg:
                    nc.vector.memset(at, 1.0)
                    return at
                ac = acp.tile([32, RCH + 2, PW], BF16, tag="ac")
                for b in range(NB):
                    nc.scalar.dma_start(
                        out=ac[NCHUNK * b:NCHUNK * b + NCHUNK, :, :],
                        in_=AP(X[:].tensor, (32 * b + 3) * XEL,
                               [[XEL, 1], [RCH * PW, NCHUNK], [1, ACEL]]))
                ab = mpp.tile([32, RCH + 2, PW], BF16, tag="ab")
                nc.vector.tensor_single_scalar(ab, ac, 0.01, OP.is_gt)
                d1 = mpp.tile([32, RCH + 2, W], BF16, tag="d1")
                nc.vector.tensor_tensor(
                    d1, ab[:, :, 1:W + 1], ab[:, :, 3:W + 3], OP.add)
                nc.vector.tensor_tensor(d1, d1, ab[:, :, 2:W + 2], OP.add)
                d2 = mpp.tile([32, RCH, W], BF16, tag="d2")
                nc.vector.tensor_tensor(
                    d2, d1[:, 0:RCH], d1[:, 2:RCH + 2], OP.add)
                nc.vector.tensor_tensor(d2, d2, d1[:, 1:RCH + 1], OP.add)
                av = avp.tile([32, CC], FP8, tag="av")
                nc.vector.tensor_single_scalar(
                    av, d2.rearrange("p r w -> p (r w)"), 0.5, OP.is_gt)
                # broadcast av -> at via DRAM bounce (partition replication
                # needs a linear source): at[32b+c, k, pix] = av[8b+k, pix]
                avd = dramp.tile([32, CC], FP8, tag="avd", name="avd")
                nc.gpsimd.dma_start(out=avd, in_=av)
                for b in range(NB):
                    nc.gpsimd.dma_start(
                        out=at[32 * b:32 * b + 32, :, :],
                        in_=avd[NCHUNK * b:NCHUNK * b + NCHUNK, :]
                            .rearrange("k c -> (k c)").unsqueeze(0)
                            .to_broadcast([32, NCHUNK * CC])
                            .rearrange("c (k p) -> c k p", k=NCHUNK))
                return at

            def epilogue(ep):
                """Pipelined chunk epilogue: xnm = xn*alive (GPSIMD), then
                Y = tanh(xnm) (ACT) + seed DMAs for chunk 2; on the final
                step also stream this chunk's rows to DRAM."""
                Y, k, xn, at, final = ep
                xnm = xmp.tile([128, CC], BF16, tag="xnm")
                at3 = at[:, k, :].rearrange("p (r w) -> p r w", r=RCH)
                xn3 = xn.rearrange("p (r w) -> p r w", r=RCH)
                if "nogps" in _dbg:
                    nc.vector.tensor_tensor(xnm[0:112], xn3[0:112].rearrange(
                        "p r w -> p (r w)"), at[0:112, k, :], OP.mult)
                else:
                    nc.gpsimd.tensor_tensor(
                        xnm.rearrange("p (r w) -> p r w", r=RCH)[0:112],
                        xn3[0:112], at3[0:112], OP.mult)
                yi = Y[0:112, 16 * k + 1:16 * k + 1 + RCH, 2:2 + W]
                nc.scalar.activation(
                    yi, xnm.rearrange("p (r w) -> p r w", r=RCH)[0:112],
                    AF.Tanh)
                if k == 2:      # seed: image (32,32) = tile (33,34)
                    for b in range(NB):
                        nc.scalar.dma_start(
                            out=Y[32 * b + 3:32 * b + CH, 33:34, 34:35],
                            in_=seedd[:].to_broadcast([CH - 3, 1]))
                if final:
                    for b in range(NB):
                        nc.gpsimd.dma_start(
                            out=xout[b, :, 16 * k:16 * k + RCH, :],
                            in_=Y[32 * b:32 * b + CH,
                                  16 * k + 1:16 * k + 1 + RCH, 2:2 + W])

            # ---------------- software-pipelined emission ----------------
            # Iteration k emits: B(k) [L2/h2ev/L3 h=0 + trio h=1],
            # xn_evac(0,k), stk-prefetch(k+1), epilogue(k-1), C(k)+A(k+1)
            # merged per image [L2(i,1); trio(i,0,k+1); h2ev; L3(i,1)],
            # xn_evac(1,k).  A(k+1) trios cover h2ev latency so PE never
            # stalls between L2 and L3; the PE stream also crosses step
            # boundaries seamlessly (A of chunk 0 next step is emitted in
            # iteration 7 of this step).
            pend = []     # pending epilogue args
            mt_next = []  # prefetched mask tile
            PWS = PW + 2               # padded stk row (no AP merges)
            SPIT = RCH * PWS           # stk partition pitch

            def build_stks(k, X):
                stks = []
                for j in range(2):
                    stk = stkp.tile([128, RCH, PWS], BF16, tag="stk")
                    for i in range(2):
                        for d in range(3):
                            nc.sync.dma_start(
                                out=AP(stk[:].tensor,
                                       (64 * i + 16 * d) * SPIT,
                                       [[SPIT, CH], [1, RCH * PW]]),
                                in_=AP(X[:].tensor,
                                       (64 * j + 32 * i) * XEL
                                       + (16 * k + d) * PW,
                                       [[XEL, CH], [1, RCH * PW]]))
                    stks.append(stk)
                return stks

            def trio(i, h, stks):
                """L1 for image i, half h: 6 matmuls + relu evac.
                Returns the [128, 1024] bf16 h1-half tile."""
                stkf = stks[i // 2].rearrange("p r w -> p (r w)")
                stk3 = AP(stkf.tensor, 0,
                          [[RCH * PWS, 128], [PW, RCH], [1, PW]])
                hb = 64 * (i % 2)
                ps1 = ps1p.tile([128, HC], F32, tag="ps1")
                for n2 in range(2):
                    r0 = 8 * h + 4 * n2
                    for t in range(3):
                        nc.tensor.matmul(
                            ps1[:, 512 * n2:512 * n2 + 512],
                            w1s[hb:hb + 48, t, :],
                            stk3[hb:hb + 48, r0:r0 + 4, t + 1:t + 1 + W],
                            start=(t == 0), stop=(t == 2))
                dst = h1p.tile([128, HC], BF16, tag="h1")
                nc.scalar.activation(dst, ps1, AF.Relu, bias=b1s)
                return dst

            def l2(h1half):
                ps2 = ps2p.tile([128, HC], F32, tag="ps2")
                for m in range(2):
                    nc.tensor.matmul(
                        ps2[:, 512 * m:512 * m + 512], w2s,
                        h1half[:, 512 * m:512 * m + 512],
                        start=True, stop=True)
                return ps2

            def h2ev(k, h, ps2, mt):
                h2m = h2p.tile([128, HC], BF16, tag="h2m")
                if fuse_mask:
                    nc.vector.scalar_tensor_tensor(
                        h2m, ps2, 0.0, mt[:, k, HC * h:HC * h + HC],
                        op0=OP.max, op1=OP.mult)
                else:
                    nc.scalar.activation(h2m, ps2, AF.Relu, bias=b2s)
                    nc.vector.tensor_tensor(
                        h2m, h2m, mt[:, k, HC * h:HC * h + HC], OP.mult)
                return h2m

            def l3(i, h2m, ps3):
                cp = 32 * i
                for m in range(2):
                    nc.tensor.matmul(
                        ps3[cp:cp + 32, 512 * m:512 * m + 512], w3s,
                        h2m[:, 512 * m:512 * m + 512],
                        start=True, stop=True, tile_position=(0, cp))

            def xn_evac(k, h, ps3, X, xn3):
                xi_h = X[0:112,
                         16 * k + 1 + 8 * h:16 * k + 1 + 8 * h + 8,
                         2:2 + W]
                nc.vector.scalar_tensor_tensor(
                    xn3[0:112, 8 * h:8 * h + 8, :],
                    ps3[0:112, :].rearrange("p (r w) -> p r w", r=8),
                    b3s[0:112, :], xi_h, op0=OP.add, op1=OP.add)

            def epilogue(ep):
                """xnm = xn*alive (GPSIMD), Y = tanh(xnm) (ACT), seed for
                chunk 2; on the final step stream this chunk to DRAM."""
                Y, k, xn, at, final = ep
                xnm = xmp.tile([128, CC], BF16, tag="xnm")
                at3 = at[:, k, :].rearrange("p (r w) -> p r w", r=RCH)
                xn3 = xn.rearrange("p (r w) -> p r w", r=RCH)
                xnm3 = xnm.rearrange("p (r w) -> p r w", r=RCH)
                nc.gpsimd.tensor_tensor(
                    xnm3[0:112], xn3[0:112], at3[0:112], OP.mult)
                yi = Y[0:112, 16 * k + 1:16 * k + 1 + RCH, 2:2 + W]
                nc.scalar.activation(yi, xnm3[0:112], AF.Tanh)
                if k == 2:      # seed: image (32,32) = tile (33,34)
                    for b in range(NB):
                        nc.scalar.dma_start(
                            out=Y[32 * b + 3:32 * b + CH, 33:34, 34:35],
                            in_=seedd[:].to_broadcast([CH - 3, 1]))
                if final:
                    for b in range(NB):
                        nc.gpsimd.dma_start(
                            out=xout[b, :, 16 * k:16 * k + RCH, :],
                            in_=Y[32 * b:32 * b + CH,
                                  16 * k + 1:16 * k + 1 + RCH, 2:2 + W])

            def step_emit(X, Y, mt, carry, Xnext=None, final=False):
                """Emit one step.  carry = [stks(ch0), h0s(ch0)] produced by
                the previous step's iteration 7 (or the manual prologue).
                Returns the carry for the next step (or None)."""
                at = None
                for k in range(NCHUNK):
                    stksK, h0s = carry
                    xn = xnp.tile([128, CC], F32, tag="xn")
                    xn3 = xn.rearrange("p (r w) -> p r w", r=RCH)
                    # ---- B(k): h=0 through the MLP + trios h=1; L3 is
                    # skewed one image so PE never waits on the DVE evac ----
                    h1s = []
                    ps3a = ps3p.tile([128, HC], F32, tag="ps3")
                    lpend = None
                    for i in range(NB):
                        ps2 = l2(h0s[i])
                        h1s.append(trio(i, 1, stksK))
                        if lpend is not None:
                            l3(*lpend)
                        lpend = (i, h2ev(k, 0, ps2, mt), ps3a)
                    l3(*lpend)
                    xn_evac(k, 0, ps3a, X, xn3)
                    # prefetch stacks for the next merged-A target
                    if k < NCHUNK - 1:
                        stks_nxt = build_stks(k + 1, X)
                    elif Xnext is not None:
                        stks_nxt = build_stks(0, Xnext)
                    else:
                        stks_nxt = None
                    # ---- C(k) merged with A(next), same skew ----
                    ps3b = ps3p.tile([128, HC], F32, tag="ps3")
                    h0s_nxt = []
                    lpend = None
                    for i in range(NB):
                        ps2 = l2(h1s[i])
                        if stks_nxt is not None:
                            h0s_nxt.append(trio(i, 0, stks_nxt))
                        if lpend is not None:
                            l3(*lpend)
                        lpend = (i, h2ev(k, 1, ps2, mt), ps3b)
                    l3(*lpend)
                    xn_evac(k, 1, ps3b, X, xn3)
                    # ---- housekeeping (after the relu stream: ACT's tanh
                    # of the previous chunk must not delay this chunk) ----
                    if pend:
                        epilogue(pend.pop())
                    if k == 0:
                        at = alive_chain(X)
                    if k == 6 and mt_next:
                        mt_next[0] = mt_bcast(*mt_next[0])
                    pend.append((Y, k, xn, at, final))
                    carry = [stks_nxt, h0s_nxt]
                return carry if carry[0] is not None else None

            def prologue_A(X):
                stks0 = build_stks(0, X)
                return [stks0, [trio(i, 0, stks0) for i in range(NB)]]

            if loop_n is not None:
                UN = 4 if loop_n % 4 == 0 else 2
                assert loop_n % UN == 0
                with tc.For_i(0, loop_n, UN, staggered_reset=True) as si:
                    mt = mt_bcast(None, (si, 0))
                    carry = prologue_A(XA)
                    cur = [XA, XB]
                    for j in range(UN):
                        Xj, Yj = cur[j % 2], cur[(j + 1) % 2]
                        mt_next.clear()
                        if j + 1 < UN:
                            mt_next.append((None, (si, j + 1)))
                        carry = step_emit(
                            Xj, Yj, mt, carry,
                            Xnext=(Yj if j + 1 < UN else None))
                        if mt_next:
                            mt = mt_next[0]
                    if pend:
                        epilogue(pend.pop())
                # timing variant: single writeout at the end
                for b in range(NB):
                    nc.gpsimd.dma_start(
                        out=xout[b, :, :, :],
                        in_=XA[32 * b:32 * b + CH, 1:H + 1, 2:2 + W])
            else:
                ping = [XA, XB]
                mt = mt_bcast(0)
                carry = prologue_A(XA)
                for si in range(steps):
                    X, Y = ping[si % 2], ping[(si + 1) % 2]
                    mt_next.clear()
                    if si + 1 < steps:
                        mt_next.append((si + 1,))
                    carry = step_emit(
                        X, Y, mt, carry,
                        Xnext=(Y if si + 1 < steps else None),
                        final=(si == steps - 1))
                    if mt_next:
                        mt = mt_next[0]
                if pend:
                    epilogue(pend.pop())

    nc.compile()
    return nc


def _prep_weights(w1, b1, w2, b2, w3, b3):
    import ml_dtypes
    bf = ml_dtypes.bfloat16
    w1 = np.asarray(w1, np.float32)
    W1x, W1gx, W1gy, W1lap = (w1[:, 0:16], w1[:, 16:32],
                              w1[:, 32:48], w1[:, 48:64])
    A = (KX[:, :, None, None] * W1gx[None, None] +
         KY[:, :, None, None] * W1gy[None, None] +
         KL[:, :, None, None] * W1lap[None, None])
    A = A.astype(np.float32).copy()
    A[1, 1] += W1x
    blk = A.transpose(0, 3, 1, 2).reshape(48, 3, HID)
    w1t = np.zeros((128, 3, HID), np.float32)
    w1t[0:48] = blk
    w1t[64:112] = blk
    w2t = np.ascontiguousarray(np.asarray(w2, np.float32).T)
    w3t = np.zeros((HID, 2 * CH), np.float32)
    w3t[:, 0:CH] = 0.05 * np.asarray(w3, np.float32).T
    b1v = np.asarray(b1, np.float32).reshape(HID, 1)
    b2v = np.asarray(b2, np.float32).reshape(HID, 1)
    b3v = np.zeros((128, 1), np.float32)
    for b in range(NB):
        b3v[32 * b:32 * b + CH, 0] = 0.05 * np.asarray(b3, np.float32)
    return (w1t.astype(bf), w2t.astype(bf), w3t.astype(bf), b1v, b2v, b3v)


LAST_EXEC_NS = None
LAST_PROFILE_JSON = None
LAST_TRACE_PATH = None


def kernel(x, rand, w1, b1, w2, b2, w3, b3, steps=STEPS):
    import ml_dtypes
    from concourse import mybir
    from concourse.bass_utils import run_bass_kernel_spmd

    import os
    fuse_mask = bool(np.all(np.asarray(b2) == 0.0))
    key = (steps, fuse_mask)
    if key not in _BUILD_CACHE:
        _BUILD_CACHE[key] = _build(steps, fuse_mask=fuse_mask)
    nc = _BUILD_CACHE[key]

    f8 = mybir.dt.np(mybir.dt.float8e4)
    x = np.asarray(x, np.float32)
    rand = np.asarray(rand, np.float32)
    w1t, w2t, w3t, b1v, b2v, b3v = _prep_weights(w1, b1, w2, b2, w3, b3)
    m8c = (rand < 0.5).astype(np.float32).reshape(STEPS * NCHUNK, CC)
    if isinstance(steps, tuple):
        reps = -(-steps[1] * NCHUNK // m8c.shape[0])
        m8c = np.tile(m8c, (reps, 1))[:steps[1] * NCHUNK]
    m8c = m8c.astype(f8)

    in_maps = []
    for c in range(NCORES):
        shard = x[c * NB:(c + 1) * NB]                 # [4, 16, H, W]
        xp = np.zeros((NB, CH, PH, PW), np.float32)
        xp[:, :, 1:H + 1, 2:2 + W] = shard
        xp = xp.astype(ml_dtypes.bfloat16)
        in_maps.append({
            "xin": xp, "m8c": m8c, "w1t": w1t, "w2t": w2t,
            "w3t": w3t, "b1d": b1v, "b2d": b2v, "b3d": b3v,
            "seedd": np.array([np.tanh(1.0)], ml_dtypes.bfloat16),
        })

    run_kwargs = {}
    if os.environ.get("NCA_TRACE"):
        run_kwargs["trace"] = True
        td = os.environ.get("NCA_TRACE_DIR")
        if td:
            os.makedirs(td, exist_ok=True)
            run_kwargs["tmpdir"] = td
    res = run_bass_kernel_spmd(nc, in_maps, core_ids=list(range(NCORES)),
                               **run_kwargs)
    if os.environ.get("NCA_TRACE"):
        global LAST_EXEC_NS, LAST_PROFILE_JSON, LAST_TRACE_PATH
        LAST_EXEC_NS = res.exec_time_ns
        LAST_PROFILE_JSON = res.profile_json
        if res.instructions_and_trace is not None:
            LAST_TRACE_PATH = res.instructions_and_trace[1]
    out = np.concatenate([r["xout"] for r in res.results], axis=0)
    return out.astype(np.float32)
